# revision 16
# baseline (speedup 1.0000x reference)
"""AttnCutLoss on 8 Trainium2 NeuronCores (pure data parallel over batch).

loss = -sum_{b,j} log(output[b,j]) * q[b,j] / B,  q = softmax_j(r/tau),
r[b,j] = F1-at-cutoff-(j+1) = 2*csum[b,j] / (j+1 + T[b])   (harmonic-mean
identity; exact also when csum==0 or T==0), with csum = cumsum_j(labels),
T = total relevant per row.

z = r/tau lies in [0, 1/tau], so softmax needs no max-subtraction:
per row, loss_b = -sum(ln(out)*e^z)/sum(e^z).

Per core: 8 tiles of [128 rows x 2048].  labels ship as uint8, output as
float16 (~5e-4 rel, washes out in the 16M-term average).  recip factor
(2/tau)/(k+T[b]) comes from a host-built table RTAB[T, j], fetched per tile
with an indirect row-gather keyed by T (gpsimd).

Engine assignment per tile (measured costs, ns):
  DVE : scan 4410, offs cast 146, z = csum*recip 1191,
        w = e*lo with fused row-sum accumulate (tensor_tensor_reduce) 1191
  ACT : lo = ln(out) 2001, e = exp(z) 2282 (+accum read 279) -> s
  GpSimd: indirect row-gather ~4400
  DMA : lab 790 + out 1579 per tile

Software-pipelined emission: scans run two tiles ahead and gathers one
tile ahead of the consume stage, so the DVE never stalls on the gather and
the steady-state period is the DVE's ~6.9 us.

Host: loss = -sum(ip/s)/B in float64, cast to float32.

The Bacc activation-table pass is pinned so Exp and Ln share one table
(natural_log_exp_and_others); the default greedy choice alternates two
tables and pays a 1.3us ACT_TABLE_LOAD per activation.
"""

import numpy as np

import bass_rust as _bass_rust
import concourse.bass as bass
import concourse.tile as tile
from concourse import bacc, mybir
from concourse.bass_utils import run_bass_kernel_spmd
from concourse.hw_specs import get_activation_tables

B, L = 8192, 2048
N_CORES = 8
ROWS_PER_CORE = B // N_CORES          # 1024
P = 128                               # SBUF partitions
NT = ROWS_PER_CORE // P               # 8 tiles per core
TAU = 0.95
VTAB = L + 1                          # T can be 0..2048

# w-pass mode: "ttr" = fused DVE tensor_tensor_reduce (mult + row-sum);
# "x" = DVE tensor_tensor mult + ACT Copy-accumulate row-sum.
W_MODE = "x"

_CACHE = {}


def _pin_act_tables(nc):
    """Per-instance override: keep Exp/Ln only in the combined table so the
    table-load pass can't alternate between the exp-only and ln-only sets."""

    def patched(self):
        has_activation = any(
            isinstance(i, mybir.InstActivation)
            for b in self.main_func.blocks
            for i in b.instructions
        )
        if not has_activation:
            return
        AF = mybir.ActivationFunctionType
        keep = "natural_log_exp_and_others"
        tables = []
        for name, funcs in get_activation_tables(self.m.arch).items():
            if name != keep:
                funcs = {f for f in funcs if f not in (AF.Exp, AF.Ln)}
            tables.append((name, funcs))
        _bass_rust.insert_act_table_loads(self, tables)

    nc.insert_act_table_loads = patched.__get__(nc)


def _build_nc():
    f16 = mybir.dt.float16
    f32 = mybir.dt.float32
    i32 = mybir.dt.int32
    u8 = mybir.dt.uint8
    AF = mybir.ActivationFunctionType
    OP = mybir.AluOpType

    nc = bacc.Bacc("TRN2", target_bir_lowering=False, debug=False)
    _pin_act_tables(nc)
    labels_d = nc.dram_tensor("labels", [ROWS_PER_CORE, L], u8, kind="ExternalInput")
    outp_d = nc.dram_tensor("outp", [ROWS_PER_CORE, L], f16, kind="ExternalInput")
    rtab_d = nc.dram_tensor("rtab", [VTAB, L], f16, kind="ExternalInput")
    ip_d = nc.dram_tensor("ip_out", [P, NT], f32, kind="ExternalOutput")
    s_d = nc.dram_tensor("s_out", [P, NT], f32, kind="ExternalOutput")

    with tile.TileContext(nc) as tc:
        with (
            tc.tile_pool(name="plab", bufs=4) as plab,
            tc.tile_pool(name="pout", bufs=4) as pout,
            tc.tile_pool(name="pcsum", bufs=3) as pcsum,
            tc.tile_pool(name="precip", bufs=4) as precip,
            tc.tile_pool(name="pz", bufs=4) as pz,
            tc.tile_pool(name="pe", bufs=3) as pe,
            tc.tile_pool(name="plo", bufs=4) as plo,
            tc.tile_pool(name="pw", bufs=4) as pw,
            tc.tile_pool(name="poffs", bufs=3) as poffs,
            tc.tile_pool(name="pres", bufs=1) as pres,
        ):
            ip_sb = pres.tile([P, NT], f32)
            s_sb = pres.tile([P, NT], f32)

            lab = [None] * NT
            out = [None] * NT
            csum = [None] * NT
            offs = [None] * NT
            recip = [None] * NT
            z = [None] * NT
            e = [None] * NT
            lo = [None] * NT

            def load(t):
                rows = slice(t * P, (t + 1) * P)
                lab[t] = plab.tile([P, L], u8, name="lab")
                nc.sync.dma_start(lab[t][:], labels_d.ap()[rows, :])
                out[t] = pout.tile([P, L], f16, name="out")
                nc.sync.dma_start(out[t][:], outp_d.ap()[rows, :])

            def scan_cast_gather(t):
                # scan on DVE; offs cast + indirect gather on gpsimd so the
                # gather's ~2.3us transfer+semaphore latency hides a full
                # pipeline period before z_t consumes recip_t.
                csum[t] = pcsum.tile([P, L], f16, name="csum")
                nc.vector.tensor_tensor_scan(
                    csum[t][:], lab[t][:], lab[t][:], 0.0, OP.add, OP.bypass
                )
                offs[t] = poffs.tile([P, 1], i32, name="offs")
                nc.vector.tensor_copy(offs[t][:], csum[t][:, L - 1 : L])
                recip[t] = precip.tile([P, L], f16, name="recip")
                # two half-width gathers halve the transfer quantum on the
                # SWDGE queue, so z_t stops waiting ~2us on the transfer
                half = L // 2
                nc.gpsimd.indirect_dma_start(
                    out=recip[t][:, 0:half],
                    out_offset=None,
                    in_=rtab_d.ap()[:, 0:half],
                    in_offset=bass.IndirectOffsetOnAxis(ap=offs[t][:, :1], axis=0),
                )
                nc.gpsimd.indirect_dma_start(
                    out=recip[t][:, half:L],
                    out_offset=None,
                    in_=rtab_d.ap(),
                    in_offset=bass.IndirectOffsetOnAxis(ap=offs[t][:, :1], axis=0),
                    element_offset=half,
                )

            # prologue; a dummy gather (offsets 0) warms the gpsimd SWDGE
            # path during the first loads - the first real gather otherwise
            # pays ~3us extra.
            def load_lab(t):
                rows = slice(t * P, (t + 1) * P)
                lab[t] = plab.tile([P, L], u8, name="lab")
                nc.sync.dma_start(lab[t][:], labels_d.ap()[rows, :])

            def load_out(t):
                rows = slice(t * P, (t + 1) * P)
                out[t] = pout.tile([P, L], f16, name="out")
                nc.sync.dma_start(out[t][:], outp_d.ap()[rows, :])

            load_lab(0)
            load_out(0)
            load_lab(1)
            load_lab(2)
            load_out(1)
            load_out(2)
            scan_cast_gather(0)
            scan_cast_gather(1)
            scan_cast_gather(2)

            def ln_pass(t):
                lo[t] = plo.tile([P, L], f16, name="lo")
                nc.scalar.activation(lo[t][:], out[t][:], AF.Ln)

            ln_pass(0)
            ln_pass(1)

            def z_pass(t):
                z[t] = pz.tile([P, L], f16, name="z")
                nc.vector.tensor_tensor(
                    out=z[t][:], in0=csum[t][:], in1=recip[t][:], op=OP.mult
                )

            def exp_pass(t):
                e[t] = pe.tile([P, L], f16, name="e")
                nc.scalar.activation(
                    e[t][:], z[t][:], AF.Exp, accum_out=s_sb[:, t : t + 1]
                )

            def w_copy(t):
                w = pw.tile([P, L], f16, name="w")
                nc.vector.tensor_tensor(
                    out=w[:], in0=e[t][:], in1=lo[t][:], op=OP.mult
                )
                wc = pw.tile([P, L], f16, name="wc")
                nc.scalar.activation(
                    wc[:], w[:], AF.Copy, accum_out=ip_sb[:, t : t + 1]
                )

            def w_stt(t):
                w = pw.tile([P, L], f16, name="w")
                nc.vector.scalar_tensor_tensor(
                    out=w[:], in0=e[t][:], scalar=1.0, in1=lo[t][:],
                    op0=OP.mult, op1=OP.mult, accum_out=ip_sb[:, t : t + 1],
                )

            # steady-state periods 0..4: scans/gathers three tiles ahead,
            # ln two ahead; DVE period = z + scan + cast + w = 6.96us paces.
            for t in range(5):
                load(t + 3)
                z_pass(t)           # first: unblocks ACT exp early
                scan_cast_gather(t + 3)
                ln_pass(t + 2)
                exp_pass(t)
                if t == 4:
                    # pull ln7 forward (fills ACT's wait for w4) so the ACT
                    # tail is exp5/exp6/exp7/copy5 back-to-back
                    ln_pass(7)
                w_copy(t)
            # tail (tiles 5-7, no scans left): hoist the z's, run all exps
            # back-to-back on ACT, finish with DVE STT row-sums so the last
            # ACT op (copy5) and DVE op (stt7) land together (~11us tail).
            z_pass(5)
            z_pass(6)
            z_pass(7)
            exp_pass(5)
            exp_pass(6)
            exp_pass(7)
            nc.sync.dma_start(s_d.ap(), s_sb[:])
            w_stt(5)
            w_stt(6)
            w_stt(7)

            nc.sync.dma_start(ip_d.ap(), ip_sb[:])
    nc.compile()
    return nc


def _get_nc():
    if "nc" not in _CACHE:
        _CACHE["nc"] = _build_nc()
    return _CACHE["nc"]


def _get_rtab():
    if "rtab" not in _CACHE:
        t = np.arange(VTAB, dtype=np.float64)[:, None]
        k = np.arange(1, L + 1, dtype=np.float64)[None, :]
        _CACHE["rtab"] = ((2.0 / TAU) / (k + t)).astype(np.float16)
    return _CACHE["rtab"]


def _make_in_maps(output, labels):
    outp = np.asarray(output, dtype=np.float32).reshape(B, L).astype(np.float16)
    lab = np.asarray(labels).astype(np.uint8)
    rtab = _get_rtab()
    in_maps = []
    for c in range(N_CORES):
        rows = slice(c * ROWS_PER_CORE, (c + 1) * ROWS_PER_CORE)
        in_maps.append(
            {
                "labels": np.ascontiguousarray(lab[rows]),
                "outp": np.ascontiguousarray(outp[rows]),
                "rtab": rtab,
            }
        )
    return in_maps


def _reduce_results(results):
    total = 0.0
    for r in results:
        ip = r["ip_out"].astype(np.float64)
        s = r["s_out"].astype(np.float64)
        total += float((ip / s).sum())
    return np.float32(-total / B)


def kernel(output, labels):
    nc = _get_nc()
    in_maps = _make_in_maps(output, labels)
    res = run_bass_kernel_spmd(nc, in_maps, list(range(N_CORES)))
    return _reduce_results(res.results)


# revision 17
# speedup vs baseline: 1.1138x; 1.1138x over previous
"""AttnCutLoss on 8 Trainium2 NeuronCores (pure data parallel over batch).

loss = -sum_{b,j} log(output[b,j]) * q[b,j] / B,  q = softmax_j(r/tau),
r[b,j] = F1-at-cutoff-(j+1) = 2*csum[b,j] / (j+1 + T[b])   (harmonic-mean
identity; exact also when csum==0 or T==0), with csum = cumsum_j(labels),
T = total relevant per row.

z = r/tau lies in [0, 1/tau], so softmax needs no max-subtraction:
per row, loss_b = -sum(ln(out)*e^z)/sum(e^z).

Per core: 8 tiles of [128 rows x 2048].  labels ship as uint8, output as
float16 (~5e-4 rel, washes out in the 16M-term average).  recip factor
(2/tau)/(k+T[b]) comes from a host-built table RTAB[T, j], fetched per tile
with an indirect row-gather keyed by T (gpsimd).

Engine assignment per tile (measured costs, ns):
  DVE : scan 4410, offs cast 146, z = csum*recip 1191,
        w = e*lo with fused row-sum accumulate (tensor_tensor_reduce) 1191
  ACT : lo = ln(out) 2001, e = exp(z) 2282 (+accum read 279) -> s
  GpSimd: indirect row-gather ~4400
  DMA : lab 790 + out 1579 per tile

Software-pipelined emission: scans run two tiles ahead and gathers one
tile ahead of the consume stage, so the DVE never stalls on the gather and
the steady-state period is the DVE's ~6.9 us.

Host: loss = -sum(ip/s)/B in float64, cast to float32.

The Bacc activation-table pass is pinned so Exp and Ln share one table
(natural_log_exp_and_others); the default greedy choice alternates two
tables and pays a 1.3us ACT_TABLE_LOAD per activation.
"""

import numpy as np

import bass_rust as _bass_rust
import concourse.bass as bass
import concourse.tile as tile
from concourse import bacc, mybir
from concourse.bass_utils import run_bass_kernel_spmd
from concourse.hw_specs import get_activation_tables

B, L = 8192, 2048
N_CORES = 8
ROWS_PER_CORE = B // N_CORES          # 1024
P = 128                               # SBUF partitions
NT = ROWS_PER_CORE // P               # 8 tiles per core
TAU = 0.95
VTAB = L + 1                          # T can be 0..2048

# w-pass mode: "ttr" = fused DVE tensor_tensor_reduce (mult + row-sum);
# "x" = DVE tensor_tensor mult + ACT Copy-accumulate row-sum.
W_MODE = "x"

_CACHE = {}


def _pin_act_tables(nc):
    """Per-instance override: keep Exp/Ln only in the combined table so the
    table-load pass can't alternate between the exp-only and ln-only sets."""

    def patched(self):
        has_activation = any(
            isinstance(i, mybir.InstActivation)
            for b in self.main_func.blocks
            for i in b.instructions
        )
        if not has_activation:
            return
        AF = mybir.ActivationFunctionType
        keep = "natural_log_exp_and_others"
        tables = []
        for name, funcs in get_activation_tables(self.m.arch).items():
            if name != keep:
                funcs = {f for f in funcs if f not in (AF.Exp, AF.Ln)}
            tables.append((name, funcs))
        _bass_rust.insert_act_table_loads(self, tables)

    nc.insert_act_table_loads = patched.__get__(nc)


def _build_nc():
    f16 = mybir.dt.float16
    f32 = mybir.dt.float32
    i32 = mybir.dt.int32
    u8 = mybir.dt.uint8
    AF = mybir.ActivationFunctionType
    OP = mybir.AluOpType

    nc = bacc.Bacc("TRN2", target_bir_lowering=False, debug=False)
    _pin_act_tables(nc)
    labels_d = nc.dram_tensor("labels", [ROWS_PER_CORE, L], u8, kind="ExternalInput")
    outp_d = nc.dram_tensor("outp", [ROWS_PER_CORE, L], f16, kind="ExternalInput")
    rtab_d = nc.dram_tensor("rtab", [VTAB, L], f16, kind="ExternalInput")
    ip_d = nc.dram_tensor("ip_out", [P, NT], f32, kind="ExternalOutput")
    s_d = nc.dram_tensor("s_out", [P, NT], f32, kind="ExternalOutput")

    with tile.TileContext(nc) as tc:
        with (
            tc.tile_pool(name="plab", bufs=4) as plab,
            tc.tile_pool(name="pout", bufs=4) as pout,
            tc.tile_pool(name="pcsum", bufs=3) as pcsum,
            tc.tile_pool(name="precip", bufs=4) as precip,
            tc.tile_pool(name="pz", bufs=4) as pz,
            tc.tile_pool(name="pe", bufs=3) as pe,
            tc.tile_pool(name="plo", bufs=4) as plo,
            tc.tile_pool(name="pw", bufs=4) as pw,
            tc.tile_pool(name="poffs", bufs=3) as poffs,
            tc.tile_pool(name="pres", bufs=1) as pres,
        ):
            ip_sb = pres.tile([P, NT], f32)
            s_sb = pres.tile([P, NT], f32)

            lab = [None] * NT
            out = [None] * NT
            csum = [None] * NT
            offs = [None] * NT
            recip = [None] * NT
            z = [None] * NT
            e = [None] * NT
            lo = [None] * NT

            def load(t):
                rows = slice(t * P, (t + 1) * P)
                lab[t] = plab.tile([P, L], u8, name="lab")
                nc.sync.dma_start(lab[t][:], labels_d.ap()[rows, :])
                out[t] = pout.tile([P, L], f16, name="out")
                nc.sync.dma_start(out[t][:], outp_d.ap()[rows, :])

            def scan_cast_gather(t):
                # scan on DVE; offs cast + indirect gather on gpsimd so the
                # gather's ~2.3us transfer+semaphore latency hides a full
                # pipeline period before z_t consumes recip_t.
                csum[t] = pcsum.tile([P, L], f16, name="csum")
                nc.vector.tensor_tensor_scan(
                    csum[t][:], lab[t][:], lab[t][:], 0.0, OP.add, OP.bypass
                )
                offs[t] = poffs.tile([P, 1], i32, name="offs")
                nc.vector.tensor_copy(offs[t][:], csum[t][:, L - 1 : L])
                recip[t] = precip.tile([P, L], f16, name="recip")
                nc.gpsimd.indirect_dma_start(
                    out=recip[t][:],
                    out_offset=None,
                    in_=rtab_d.ap(),
                    in_offset=bass.IndirectOffsetOnAxis(ap=offs[t][:, :1], axis=0),
                )

            # prologue; a dummy gather (offsets 0) warms the gpsimd SWDGE
            # path during the first loads - the first real gather otherwise
            # pays ~3us extra.
            def load_lab(t):
                rows = slice(t * P, (t + 1) * P)
                lab[t] = plab.tile([P, L], u8, name="lab")
                nc.sync.dma_start(lab[t][:], labels_d.ap()[rows, :])

            def load_out(t):
                rows = slice(t * P, (t + 1) * P)
                out[t] = pout.tile([P, L], f16, name="out")
                nc.sync.dma_start(out[t][:], outp_d.ap()[rows, :])

            load_lab(0)
            load_out(0)
            load_lab(1)
            load_lab(2)
            load_out(1)
            load_out(2)
            scan_cast_gather(0)
            scan_cast_gather(1)
            scan_cast_gather(2)

            def ln_pass(t):
                lo[t] = plo.tile([P, L], f16, name="lo")
                nc.scalar.activation(lo[t][:], out[t][:], AF.Ln)

            ln_pass(0)
            ln_pass(1)

            def z_pass(t):
                z[t] = pz.tile([P, L], f16, name="z")
                nc.vector.tensor_tensor(
                    out=z[t][:], in0=csum[t][:], in1=recip[t][:], op=OP.mult
                )

            def exp_pass(t):
                e[t] = pe.tile([P, L], f16, name="e")
                nc.scalar.activation(
                    e[t][:], z[t][:], AF.Exp, accum_out=s_sb[:, t : t + 1]
                )

            def w_copy(t):
                w = pw.tile([P, L], f16, name="w")
                nc.vector.tensor_tensor(
                    out=w[:], in0=e[t][:], in1=lo[t][:], op=OP.mult
                )
                wc = pw.tile([P, L], f16, name="wc")
                nc.scalar.activation(
                    wc[:], w[:], AF.Copy, accum_out=ip_sb[:, t : t + 1]
                )

            def w_stt(t):
                w = pw.tile([P, L], f16, name="w")
                nc.vector.scalar_tensor_tensor(
                    out=w[:], in0=e[t][:], scalar=1.0, in1=lo[t][:],
                    op0=OP.mult, op1=OP.mult, accum_out=ip_sb[:, t : t + 1],
                )

            # steady-state periods 0..4: scans/gathers three tiles ahead,
            # ln two ahead; DVE period = z + scan + cast + w = 6.96us paces.
            for t in range(5):
                load(t + 3)
                z_pass(t)           # first: unblocks ACT exp early
                scan_cast_gather(t + 3)
                ln_pass(t + 2)
                exp_pass(t)
                if t == 4:
                    # pull ln7 forward (fills ACT's wait for w4) so the ACT
                    # tail is exp5/exp6/exp7/copy5 back-to-back
                    ln_pass(7)
                w_copy(t)
            # tail (tiles 5-7, no scans left): hoist the z's, run all exps
            # back-to-back on ACT, finish with DVE STT row-sums so the last
            # ACT op (copy5) and DVE op (stt7) land together (~11us tail).
            z_pass(5)
            z_pass(6)
            z_pass(7)
            exp_pass(5)
            exp_pass(6)
            exp_pass(7)
            nc.sync.dma_start(s_d.ap(), s_sb[:])
            w_stt(5)
            w_stt(6)
            w_stt(7)

            nc.sync.dma_start(ip_d.ap(), ip_sb[:])
    nc.compile()
    return nc


def _get_nc():
    if "nc" not in _CACHE:
        _CACHE["nc"] = _build_nc()
    return _CACHE["nc"]


def _get_rtab():
    if "rtab" not in _CACHE:
        t = np.arange(VTAB, dtype=np.float64)[:, None]
        k = np.arange(1, L + 1, dtype=np.float64)[None, :]
        _CACHE["rtab"] = ((2.0 / TAU) / (k + t)).astype(np.float16)
    return _CACHE["rtab"]


def _make_in_maps(output, labels):
    outp = np.asarray(output, dtype=np.float32).reshape(B, L).astype(np.float16)
    lab = np.asarray(labels).astype(np.uint8)
    rtab = _get_rtab()
    in_maps = []
    for c in range(N_CORES):
        rows = slice(c * ROWS_PER_CORE, (c + 1) * ROWS_PER_CORE)
        in_maps.append(
            {
                "labels": np.ascontiguousarray(lab[rows]),
                "outp": np.ascontiguousarray(outp[rows]),
                "rtab": rtab,
            }
        )
    return in_maps


def _reduce_results(results):
    total = 0.0
    for r in results:
        ip = r["ip_out"].astype(np.float64)
        s = r["s_out"].astype(np.float64)
        total += float((ip / s).sum())
    return np.float32(-total / B)


def kernel(output, labels):
    nc = _get_nc()
    in_maps = _make_in_maps(output, labels)
    res = run_bass_kernel_spmd(nc, in_maps, list(range(N_CORES)))
    return _reduce_results(res.results)
